# revision 1
# baseline (speedup 1.0000x reference)
"""GQA attention kernel for Trainium2, 8 NeuronCores.

Sharding: data-parallel over batch (B=2) x tensor-parallel over KV heads
(HKV=4) -> 8 cores.  Core c handles batch b=c//4, kv-head j=c%4 with its
G=4 query heads.  out_proj is row-parallel; partials are reduced on host.

Layout strategy: everything transposed ([feature, seq]) so that
projections, scores and PV matmuls all consume operands natively:
  qT/kT from proj (lhsT=W, rhs=hiddenT),
  scoresT[j,i] (lhsT=kT chunk, rhs=qT),
  PV (lhsT=v natural [s,d] with a ones column -> softmax denominator),
  out_projT (lhsT=oT, rhs=Wo rows).
Softmax skips max-subtraction: q,k are rmsnorm'd so |q.k/8| <= 8 and
exp() is safe in fp32 for any non-positive mask.
"""

import numpy as np
import ml_dtypes

import concourse.bacc as bacc
import concourse.mybir as mybir
from concourse import bass_isa
from concourse.tile import TileContext
from concourse.bass_utils import run_bass_kernel_spmd

BF16 = mybir.dt.bfloat16
F32 = mybir.dt.float32
AL = mybir.AluOpType

B, S, HID = 2, 2048, 1024
H, HKV, D = 16, 4, 64
G = H // HKV  # 4 query heads per kv head
QSEL = 2 * G * D  # 512: own 256 cols + rope-partner 256 cols
ROPE_BASE = 10000.0
EPS = float(np.finfo(np.float32).eps)

NB = ml_dtypes.bfloat16

_cache: dict = {}


def _build(use_mask: bool, debug: bool = False):
    nc = bacc.Bacc("TRN2", target_bir_lowering=False)

    hT = nc.dram_tensor("hT", [8, 128, S], BF16, kind="ExternalInput")
    wq = nc.dram_tensor("wq", [8, 128, QSEL], BF16, kind="ExternalInput")
    wk = nc.dram_tensor("wk", [8, 128, 128], BF16, kind="ExternalInput")
    wv = nc.dram_tensor("wv", [8, 128, 64], BF16, kind="ExternalInput")
    wo = nc.dram_tensor("wo", [2, 128, HID], BF16, kind="ExternalInput")
    qco = nc.dram_tensor("qco", [2, 128, S], BF16, kind="ExternalInput")
    qsi = nc.dram_tensor("qsi", [2, 128, S], BF16, kind="ExternalInput")
    kco = nc.dram_tensor("kco", [64, S], BF16, kind="ExternalInput")
    ksi = nc.dram_tensor("ksi", [64, S], BF16, kind="ExternalInput")
    bqv = nc.dram_tensor("bqv", [4, 128, 1], F32, kind="ExternalInput")
    bkv = nc.dram_tensor("bkv", [2, 64, 1], F32, kind="ExternalInput")
    bvv = nc.dram_tensor("bvv", [128, 1], F32, kind="ExternalInput")
    mk = (
        nc.dram_tensor("mk", [16, 128, S], F32, kind="ExternalInput")
        if use_mask
        else None
    )
    y = nc.dram_tensor("y", [16, 128, HID], F32, kind="ExternalOutput")
    if debug:
        d_qr = nc.dram_tensor("d_qr", [128, 2, S], BF16, kind="ExternalOutput")
        d_qn = nc.dram_tensor("d_qn", [128, 2, S], BF16, kind="ExternalOutput")
        d_kn = nc.dram_tensor("d_kn", [128, S], BF16, kind="ExternalOutput")
        d_va = nc.dram_tensor("d_va", [128, 16, 66], BF16, kind="ExternalOutput")
        d_pt = nc.dram_tensor("d_pt", [128, 4, 512], BF16, kind="ExternalOutput")
        d_on = nc.dram_tensor("d_on", [128, 2, S], BF16, kind="ExternalOutput")

    with TileContext(nc) as tc:
        with tc.tile_pool(name="const", bufs=1) as cp:
            # ---- persistent tiles -------------------------------------
            wo_sb = cp.tile([128, 2, HID], BF16)
            for cc in range(2):
                nc.sync.dma_start(out=wo_sb[:, cc, :], in_=wo[cc])
            bv_sb = cp.tile([128, 1], F32)
            nc.sync.dma_start(out=bv_sb[:], in_=bvv[:])

            qr = cp.tile([128, 2, S], BF16)   # rope'd q (own 256 rows)
            qn = cp.tile([128, 2, S], BF16)   # rmsnorm'd q
            kr = cp.tile([64, S], BF16)
            kn2 = cp.tile([128, S], BF16)     # rmsnorm'd k, duplicated rows
            v_all = cp.tile([128, 16, 66], BF16)  # v natural + ones col
            oTn = cp.tile([128, 2, S], BF16)  # normalized attn out (oT)
            eps_sb = cp.tile([128, 1], F32)
            nc.vector.memset(eps_sb[:], EPS)

            # ---- phase 1: projections + rope (pools close after) ------
            with (
                tc.tile_pool(name="projc", bufs=1) as pjc,
                tc.tile_pool(name="ropetmp", bufs=4) as rtp,
                tc.tile_pool(name="stats", bufs=1) as stp,
                tc.tile_pool(name="pproj", bufs=1, space="PSUM") as pp,
            ):
                hT_sb = pjc.tile([128, 8, S], BF16)
                for ko in range(8):
                    nc.sync.dma_start(out=hT_sb[:, ko, :], in_=hT[ko])
                wq_sb = pjc.tile([128, 8, QSEL], BF16)
                wk_sb = pjc.tile([128, 8, 128], BF16)
                wv_sb = pjc.tile([128, 8, 64], BF16)
                for ko in range(8):
                    nc.sync.dma_start(out=wq_sb[:, ko, :], in_=wq[ko])
                    nc.sync.dma_start(out=wk_sb[:, ko, :], in_=wk[ko])
                    nc.sync.dma_start(out=wv_sb[:, ko, :], in_=wv[ko])
                qco_sb = pjc.tile([128, 2, S], BF16)
                qsi_sb = pjc.tile([128, 2, S], BF16)
                for cc in range(2):
                    nc.sync.dma_start(out=qco_sb[:, cc, :], in_=qco[cc])
                    nc.sync.dma_start(out=qsi_sb[:, cc, :], in_=qsi[cc])
                kco_sb = pjc.tile([64, S], BF16)
                ksi_sb = pjc.tile([64, S], BF16)
                nc.sync.dma_start(out=kco_sb[:], in_=kco[:])
                nc.sync.dma_start(out=ksi_sb[:], in_=ksi[:])
                bq_sb = pjc.tile([128, 4, 1], F32)
                for co in range(4):
                    nc.sync.dma_start(out=bq_sb[:, co, :], in_=bqv[co])
                bk_sb = pjc.tile([64, 2, 1], F32)
                for t in range(2):
                    nc.sync.dma_start(out=bk_sb[:, t, :], in_=bkv[t])
                for so in range(4):
                    sl = slice(so * 512, (so + 1) * 512)
                    psq = pp.tile([128, 4, 512], F32, tag="psq")
                    psk = pp.tile([64, 2, 512], F32, tag="psk")
                    for ko in range(8):
                        st, sp = ko == 0, ko == 7
                        for co in range(4):
                            nc.tensor.matmul(
                                psq[:, co, :],
                                lhsT=wq_sb[:, ko, co * 128:(co + 1) * 128],
                                rhs=hT_sb[:, ko, sl],
                                start=st, stop=sp,
                            )
                        for t in range(2):
                            nc.tensor.matmul(
                                psk[:, t, :],
                                lhsT=wk_sb[:, ko, t * 64:(t + 1) * 64],
                                rhs=hT_sb[:, ko, sl],
                                start=st, stop=sp,
                            )
                    # rope: roped = (own + b_own)*cos + (partner + b_par)*sin
                    for co in range(2):
                        t1 = rtp.tile([128, 512], F32, tag="t1")
                        t2 = rtp.tile([128, 512], F32, tag="t2")
                        nc.vector.scalar_tensor_tensor(
                            t1, psq[:, co, :], bq_sb[:, co, :],
                            qco_sb[:, co, sl], AL.add, AL.mult,
                        )
                        nc.vector.scalar_tensor_tensor(
                            t2, psq[:, co + 2, :], bq_sb[:, co + 2, :],
                            qsi_sb[:, co, sl], AL.add, AL.mult,
                        )
                        nc.vector.tensor_tensor(qr[:, co, sl], t1, t2, AL.add)
                    t1 = rtp.tile([64, 512], F32, tag="t1k")
                    t2 = rtp.tile([64, 512], F32, tag="t2k")
                    nc.vector.scalar_tensor_tensor(
                        t1, psk[:, 0, :], bk_sb[:, 0, :],
                        kco_sb[:, sl], AL.add, AL.mult,
                    )
                    nc.vector.scalar_tensor_tensor(
                        t2, psk[:, 1, :], bk_sb[:, 1, :],
                        ksi_sb[:, sl], AL.add, AL.mult,
                    )
                    nc.vector.tensor_tensor(kr[:, sl], t1, t2, AL.add)

                # v projection (natural layout) + ones column
                for sc in range(16):
                    psv = pp.tile([128, 64], F32, tag="psv")
                    for ko in range(8):
                        nc.tensor.matmul(
                            psv[:],
                            lhsT=hT_sb[:, ko, sc * 128:(sc + 1) * 128],
                            rhs=wv_sb[:, ko, :],
                            start=(ko == 0), stop=(ko == 7),
                        )
                    nc.vector.tensor_copy(v_all[:, sc, 0:64], psv[:])
                    nc.vector.memset(v_all[:, sc, 64:65], 1.0)

                # ---- phase 2: rmsnorm over each head's 64 dims ------------
                # all ops partition-aligned; the two heads sharing a 128-row
                # chunk are reduced separately (channels=64) but share the
                # sqrt/recip/mul ops.
                # gpsimd partition ops only behave at partition base 0 on HW:
                # odd-parity rows are DMA-shifted to base 0 for the reduce and
                # the resulting scale is DMA-shifted back up.
                def rmsnorm(s_in, s_out, npart):
                    sq = stp.tile([128, S], F32, tag="sq", name="sq")[:npart]
                    rn = stp.tile([128, S], F32, tag="rn", name="rn")[:npart]
                    nc.vector.tensor_tensor(sq, s_in, s_in, AL.mult)
                    ss_e = stp.tile([64, S], F32, tag="ss_e", name="ss_e")
                    nc.gpsimd.partition_all_reduce(
                        ss_e, sq[0:64, :], channels=64,
                        reduce_op=bass_isa.ReduceOp.add,
                    )
                    nc.scalar.activation(
                        rn[0:64, :], ss_e, mybir.ActivationFunctionType.Sqrt,
                        bias=eps_sb[0:64], scale=1.0 / 64.0,
                    )
                    nc.vector.reciprocal(rn[0:64, :], rn[0:64, :])
                    if npart == 128:
                        sq2 = stp.tile([64, S], F32, tag="sq2", name="sq2")
                        nc.sync.dma_start(out=sq2, in_=sq[64:128, :])
                        ss_o = stp.tile([64, S], F32, tag="ss_o", name="ss_o")
                        nc.gpsimd.partition_all_reduce(
                            ss_o, sq2, channels=64,
                            reduce_op=bass_isa.ReduceOp.add,
                        )
                        rno = stp.tile([64, S], F32, tag="rno", name="rno")
                        nc.scalar.activation(
                            rno, ss_o, mybir.ActivationFunctionType.Sqrt,
                            bias=eps_sb[0:64], scale=1.0 / 64.0,
                        )
                        nc.vector.reciprocal(rno, rno)
                        nc.sync.dma_start(out=rn[64:128, :], in_=rno)
                    nc.vector.tensor_tensor(s_out, s_in, rn, AL.mult)

                for ch in range(2):
                    rmsnorm(qr[:, ch, :], qn[:, ch, :], 128)
                rmsnorm(kr[:], kn2[0:64, :], 64)
                # duplicate kn rows so scores lhsT base can match qn rows
                nc.sync.dma_start(out=kn2[64:128, :], in_=kn2[0:64, :])
                if debug:
                    nc.sync.dma_start(out=d_qr[:], in_=qr[:])
                    nc.sync.dma_start(out=d_qn[:], in_=qn[:])
                    nc.sync.dma_start(out=d_kn[:], in_=kn2[:])
                    nc.sync.dma_start(out=d_va[:], in_=v_all[:])

            with (
                tc.tile_pool(name="probs", bufs=3) as ppool,
                tc.tile_pool(name="bcast", bufs=1) as bcp,
                tc.tile_pool(name="ysb", bufs=2) as ypool,
                tc.tile_pool(name="mtile", bufs=3) as mpool,
            ):
                # ---- phase 3: attention (flash over i-chunks) -------------
                with (
                    tc.tile_pool(name="pscore", bufs=2, space="PSUM") as psc,
                    tc.tile_pool(name="pacc", bufs=1, space="PSUM") as pac,
                ):
                    for ic in range(4):
                        isl = slice(ic * 512, (ic + 1) * 512)
                        pso = pac.tile([65, 4, 512], F32, tag="pso")
                        for jc in range(16):
                            pT = ppool.tile([128, 4, 512], BF16, tag="pT")
                            if use_mask:
                                mkt = mpool.tile([128, 512], F32, tag="mkt")
                                nc.sync.dma_start(out=mkt[:], in_=mk[jc][:, isl])
                            for pair in range(2):
                                pss = psc.tile([128, 2, 512], F32, tag="pss")
                                for hh in range(2):
                                    hd = pair * 2 + hh
                                    qrows = slice(64 * (hd % 2), 64 * (hd % 2) + 64)
                                    nc.tensor.matmul(
                                        pss[:, hh, :],
                                        lhsT=kn2[qrows, jc * 128:(jc + 1) * 128],
                                        rhs=qn[qrows, hd // 2, isl],
                                        start=True, stop=True,
                                    )
                                if use_mask:
                                    sm = mpool.tile([128, 2, 512], F32, tag="sm")
                                    nc.vector.scalar_tensor_tensor(
                                        sm, pss[:], 0.125,
                                        mkt[:, None, :].to_broadcast((128, 2, 512)),
                                        AL.mult, AL.add,
                                    )
                                    nc.scalar.activation(
                                        pT[:, pair * 2:pair * 2 + 2, :], sm,
                                        mybir.ActivationFunctionType.Exp,
                                    )
                                else:
                                    nc.scalar.activation(
                                        pT[:, pair * 2:pair * 2 + 2, :], pss,
                                        mybir.ActivationFunctionType.Exp,
                                        scale=0.125,
                                    )
                            if debug and ic == 0 and jc == 0:
                                nc.sync.dma_start(out=d_pt[:], in_=pT[:])
                            for hd in range(4):
                                nc.tensor.matmul(
                                    pso[:, hd, :],
                                    lhsT=v_all[:, jc, 0:65],
                                    rhs=pT[:, hd, :],
                                    start=(jc == 0), stop=(jc == 15),
                                )
                        # normalize: recip of denominator row, broadcast over
                        # the 64 head dims, multiply.  odd heads are written at
                        # partition base 0 then DMA'd to rows 64-127.
                        rcp = bcp.tile([65, 4, 512], F32, tag="rcp")
                        nc.vector.reciprocal(rcp[64:65, :, :], pso[64:65, :, :])
                        rcp0 = bcp.tile([1, 4, 512], F32, tag="rcp0")
                        nc.sync.dma_start(out=rcp0, in_=rcp[64:65, :, :])
                        for hd in range(4):
                            rb = bcp.tile([64, 512], F32, tag="rb")
                            nc.gpsimd.partition_broadcast(
                                rb, rcp0[0:1, hd, :], channels=64
                            )
                            if hd % 2 == 0:
                                nc.vector.tensor_tensor(
                                    oTn[0:64, hd // 2, isl], pso[0:64, hd, :], rb, AL.mult
                                )
                            else:
                                ood = bcp.tile([64, 512], BF16, tag="ood")
                                nc.vector.tensor_tensor(
                                    ood, pso[0:64, hd, :], rb, AL.mult
                                )
                                nc.sync.dma_start(
                                    out=oTn[64:128, hd // 2, isl], in_=ood
                                )

                # ---- phase 4: v-projection bias (zero in practice) --------
                for ch in range(2):
                    nc.vector.tensor_scalar_add(oTn[:, ch, :], oTn[:, ch, :], bv_sb[:])
                if debug:
                    nc.sync.dma_start(out=d_on[:], in_=oTn[:])

                # ---- phase 5: out_proj (row-parallel partial) -------------
                with tc.tile_pool(name="py", bufs=2, space="PSUM") as pyp:
                    for sc in range(16):
                        ssl = slice(sc * 128, (sc + 1) * 128)
                        y_sb = ypool.tile([128, HID], F32, tag="ysb")
                        for ec in range(2):
                            psy = pyp.tile([128, 512], F32, tag="psy")
                            for cc in range(2):
                                nc.tensor.matmul(
                                    psy[:],
                                    lhsT=oTn[:, cc, ssl],
                                    rhs=wo_sb[:, cc, ec * 512:(ec + 1) * 512],
                                    start=(cc == 0), stop=(cc == 1),
                                )
                            if ec == 0:
                                nc.scalar.copy(y_sb[:, 0:512], psy[:])
                            else:
                                nc.vector.tensor_copy(y_sb[:, 512:1024], psy[:])
                        nc.sync.dma_start(out=y[sc], in_=y_sb[:])

    nc.compile()
    return nc


def _get(use_mask: bool):
    if use_mask not in _cache:
        _cache[use_mask] = _build(use_mask)
    return _cache[use_mask]


def _host_prep(hidden_state, attention_mask, Wq, bq, Wk, bk, Wv, bv, Wo, use_mask):
    """Build the 8 per-core input maps."""
    half_q, half_k = HID // 2, (HKV * D) // 2  # 512, 128
    inv_q = ROPE_BASE ** (-np.arange(half_q, dtype=np.float64) / half_q)
    inv_k = ROPE_BASE ** (-np.arange(half_k, dtype=np.float64) / half_k)
    s_idx = np.arange(S, dtype=np.float64)
    ang_q = inv_q[:, None] * s_idx[None, :]  # [512, S] freq-major
    ang_k = inv_k[:, None] * s_idx[None, :]  # [128, S]
    cos_q, sin_q = np.cos(ang_q), np.sin(ang_q)
    cos_k, sin_k = np.cos(ang_k), np.sin(ang_k)

    in_maps = []
    for core in range(8):
        b, j = core // 4, core % 4
        own_q = np.arange(j * 256, (j + 1) * 256)
        par_q = own_q + 512 if j < 2 else own_q - 512
        fidx_q = own_q if j < 2 else own_q - 512
        sign = -1.0 if j < 2 else 1.0
        own_k = np.arange(j * 64, (j + 1) * 64)
        par_k = own_k + 128 if j < 2 else own_k - 128
        fidx_k = own_k if j < 2 else own_k - 128

        hTc = np.ascontiguousarray(hidden_state[b].T).astype(NB).reshape(8, 128, S)
        wq_c = np.concatenate([Wq[:, own_q], Wq[:, par_q]], axis=1)
        wq_c = wq_c.astype(NB).reshape(8, 128, QSEL)
        wk_c = np.concatenate([Wk[:, own_k], Wk[:, par_k]], axis=1)
        wk_c = wk_c.astype(NB).reshape(8, 128, 128)
        wv_c = Wv[:, own_k].astype(NB).reshape(8, 128, 64)
        wo_c = Wo[j * 256:(j + 1) * 256, :].astype(NB).reshape(2, 128, HID)
        qco_c = cos_q[fidx_q].astype(NB).reshape(2, 128, S)
        qsi_c = (sign * sin_q[fidx_q]).astype(NB).reshape(2, 128, S)
        kco_c = cos_k[fidx_k].astype(NB)
        ksi_c = (sign * sin_k[fidx_k]).astype(NB)
        bq_c = np.concatenate([bq[own_q], bq[par_q]]).astype(np.float32)
        bq_c = bq_c.reshape(4, 128, 1)
        bk_c = np.concatenate([bk[own_k], bk[par_k]]).astype(np.float32)
        bk_c = bk_c.reshape(2, 64, 1)
        bv_c = np.tile(bv[own_k], 2).astype(np.float32).reshape(128, 1)

        m = {
            "hT": hTc, "wq": wq_c, "wk": wk_c, "wv": wv_c, "wo": wo_c,
            "qco": qco_c, "qsi": qsi_c, "kco": kco_c, "ksi": ksi_c,
            "bqv": bq_c, "bkv": bk_c, "bvv": bv_c,
        }
        if use_mask:
            mT = np.ascontiguousarray(attention_mask[b].T).astype(np.float32)
            m["mk"] = mT.reshape(16, 128, S)
        in_maps.append(m)
    return in_maps


def kernel(hidden_state, attention_mask, Wq, bq, Wk, bk, Wv, bv, Wo, bo):
    hidden_state = np.asarray(hidden_state, dtype=np.float32)
    attention_mask = np.asarray(attention_mask, dtype=np.float32)
    use_mask = bool(np.any(attention_mask))
    nc = _get(use_mask)
    in_maps = _host_prep(
        hidden_state, attention_mask,
        np.asarray(Wq, np.float32), np.asarray(bq, np.float32),
        np.asarray(Wk, np.float32), np.asarray(bk, np.float32),
        np.asarray(Wv, np.float32), np.asarray(bv, np.float32),
        np.asarray(Wo, np.float32), use_mask,
    )
    res = run_bass_kernel_spmd(nc, in_maps, list(range(8)))
    out = np.zeros((B, S, HID), dtype=np.float32)
    for core in range(8):
        out[core // 4] += res.results[core]["y"].reshape(S, HID)
    out += np.asarray(bo, np.float32)[None, None, :]
    return out

